# revision 1
# baseline (speedup 1.0000x reference)
"""Multi-head causal attention (nn_Attention_29583734734990) on 8 Trainium2 cores.

Sharding: core c -> batch b = c//2, head half hh = c%2 (8 of 16 heads, as 4
head-pairs). Each core computes its partial output sum_{h in its 8 heads}
softmax(QK^T/sqrt(d), causal) V W_o[h] for its batch; the host adds the two
half-head partials per batch.

Math layout (all matmuls fp32r = full-rate fp32-in, ~1.5e-4 rounding):
  residT[m, s]   : resid transposed once on the PE (fp32 transpose mode)
  Q^T/K^T/V^T    : [2h*64, s] = W^T residT, head pair packed on partitions
  S^T tile       : [k 128, q 512] = K_h Q_h^T (K=64 matmuls, both heads via
                   partition-base 0/64 row groups)
  P^T            : exp(S^T/8) on ACT straight PSUM->SBUF(fp32r); causal zeroing
                   via gpsimd affine_select on the <=diagonal column range
  Z^T_ext        : [65, q] = [V_h | 1]^T P^T accumulated over k blocks; row 64
                   is the softmax denominator (rides free in the same matmul)
  normalize      : reciprocal of row 64, K=1 matmul broadcasts it across
                   partitions, DVE multiply; head1 lands on partitions 64:128
                   via an SBUF->SBUF fp32r DMA
  out            : [q 128, m 512] = Z2h^T.T @ Wo2h, K=128 contracts both heads
                   of a pair at once, PSUM-accumulated over the 4 pairs
"""
from contextlib import ExitStack

import numpy as np

import concourse.bass as bass
import concourse.mybir as mybir
import concourse.tile as tile
from concourse.bass_utils import run_bass_kernel_spmd
from concourse.masks import make_identity

FP32 = mybir.dt.float32
FP32R = mybir.dt.float32r
EXP = mybir.ActivationFunctionType.Exp

B, S, M, D, H = 4, 2048, 1024, 64, 16
P = 128
NP = 4          # head pairs per core
MC = M // P     # 8  m chunks
KB = S // P     # 16 k blocks
QC = S // 512   # 4  q chunks


def _split_multiwait_instructions(nc):
    """This walrus build rejects instructions carrying >1 sem-wait ("Too many
    sync wait commands"). Move extra waits onto single-wait NoOps inserted just
    before on the same engine queue (identical semantics)."""
    ctr = 0
    for fn in nc.m.functions:
        for bb in fn.blocks:
            new = []
            for inst in list(bb.instructions):
                si = inst.sync_info
                if si is not None and len(si.on_wait) > 1:
                    waits = list(si.on_wait)
                    for w in waits[:-1]:
                        ctr += 1
                        new.append(
                            mybir.InstNoOp(
                                name=f"I-splitw-{ctr}",
                                engine=inst.engine,
                                bass_nofuse=True,
                                sync_info=mybir.SyncInfo(on_wait=[w], on_update=[]),
                            )
                        )
                    inst.sync_info = mybir.SyncInfo(
                        on_wait=[waits[-1]], on_update=list(si.on_update)
                    )
                new.append(inst)
            bb.instructions = new
    return ctr


def _body(tc, nc, resid_d, wq_d, wk_d, wv_d, wo_d, out_d):
    with ExitStack() as ctx:
        const = ctx.enter_context(tc.tile_pool(name="const", bufs=1))
        ident = const.tile([P, P], FP32, name="ident")
        make_identity(nc, ident[:])
        ones_f = const.tile([P, 1], FP32, name="ones_f")
        nc.vector.memset(ones_f[:], 1.0)

        big = ctx.enter_context(tc.tile_pool(name="big", bufs=4))
        residT = [
            big.tile([P, MC, 512], FP32R, tag="residT", name=f"residT{g}")
            for g in range(4)
        ]

        z_pool = ctx.enter_context(tc.tile_pool(name="zsb", bufs=NP))
        wo_pool = ctx.enter_context(tc.tile_pool(name="wop", bufs=NP))
        wf_pool = ctx.enter_context(tc.tile_pool(name="wf", bufs=1))
        wr_pool = ctx.enter_context(tc.tile_pool(name="wr", bufs=4))

        def load_pair_weights(p):
            w_rs = []
            for w_d in (wq_d, wk_d, wv_d):
                stg = wf_pool.tile([P, MC, 2, D], FP32, tag="wf", name="stg")
                for h in range(2):
                    nc.sync.dma_start(
                        stg[:, :, h, :],
                        w_d[2 * p + h].rearrange("(mc pp) d -> pp mc d", pp=P),
                    )
                wr = wr_pool.tile([P, MC, 2, D], FP32R, tag="wr", name="wr")
                nc.vector.tensor_copy(wr[:], stg[:])
                w_rs.append(wr)
            wo_stg = wf_pool.tile([P, M], FP32, tag="wof", name="wo_stg")
            nc.sync.dma_start(
                wo_stg[:], wo_d[2 * p:2 * p + 2].rearrange("h d m -> (h d) m")
            )
            wo_r = wo_pool.tile([P, M], FP32R, tag="wo", name="wo_r")
            nc.vector.tensor_copy(wo_r[:], wo_stg[:])
            return w_rs + [wo_r]

        # prefetch pair 0's weights so its projections start right after phase 0
        pair0_w = load_pair_weights(0)

        # ---------- Phase 0: resid -> residT (PE transpose, 128x128 blocks)
        with (
            tc.tile_pool(name="rs", bufs=8) as rs_pool,
            tc.tile_pool(name="tp", bufs=2, space="PSUM") as tp_pool,
        ):
            for sg in range(4):
                rss = []
                for sci in range(4):
                    sc = sg * 4 + sci
                    t = rs_pool.tile([P, M], FP32, tag="rs")
                    eng = (nc.sync, nc.scalar)[sc % 2]
                    eng.dma_start(t[:], resid_d[sc * P:(sc + 1) * P, :])
                    rss.append(t)
                for mi2 in range(MC // 2):
                    tp = tp_pool.tile([P, 1024], FP32, tag="tp")
                    for half in range(2):
                        mi = mi2 * 2 + half
                        for sci in range(4):
                            nc.tensor.transpose(
                                tp[:, half * 512 + sci * P:
                                   half * 512 + (sci + 1) * P],
                                rss[sci][:, mi * P:(mi + 1) * P],
                                ident[:],
                            )
                    nc.vector.tensor_copy(
                        residT[sg][:, mi2 * 2:mi2 * 2 + 2, :],
                        tp[:].rearrange("pp (mi s) -> pp mi s", mi=2),
                    )

        z_sbs, wo_rs = [], []
        with (
            tc.tile_pool(name="proj", bufs=2) as proj_pool,
            tc.tile_pool(name="projv", bufs=1) as projv_pool,
            tc.tile_pool(name="vx", bufs=1) as vx_pool,
            tc.tile_pool(name="pt", bufs=3) as pt_pool,
            tc.tile_pool(name="ztm", bufs=1) as zt_pool,
            tc.tile_pool(name="rc", bufs=2) as rc_pool,
            tc.tile_pool(name="psw", bufs=2, space="PSUM") as ps_work,
            tc.tile_pool(name="pspj", bufs=1, space="PSUM") as ps_proj,
            tc.tile_pool(name="psz", bufs=2, space="PSUM") as ps_z,
        ):
            for p in range(NP):
                wq_r, wk_r, wv_r, wo_r = (
                    pair0_w if p == 0 else load_pair_weights(p)
                )
                wo_rs.append(wo_r)

                # ---------- projections: [2h*64, s] = W2h^T @ residT
                QT = proj_pool.tile([P, S], FP32R, tag="qt")
                KT = proj_pool.tile([P, S], FP32R, tag="kt")
                VT = projv_pool.tile([P, S], FP32R, tag="vt")
                for wr, T in ((wq_r, QT), (wk_r, KT), (wv_r, VT)):
                    for sjj in range(QC // 2):
                        ps = ps_proj.tile([P, 1024], FP32, tag="w2")
                        for half in range(2):
                            sj = sjj * 2 + half
                            for mi in range(MC):
                                nc.tensor.matmul(
                                    ps[:, half * 512:(half + 1) * 512],
                                    wr[:, mi].rearrange("pp h d -> pp (h d)"),
                                    residT[sj][:, mi, :],
                                    start=(mi == 0),
                                    stop=(mi == MC - 1),
                                )
                        nc.vector.tensor_copy(
                            T[:, sjj * 1024:(sjj + 1) * 1024], ps[:]
                        )

                # ---------- V natural layout + ones column: [k, 2, 65]
                vx = vx_pool.tile([P, KB, 2, D + 1], FP32R, tag="vx")
                nc.vector.tensor_copy(
                    vx[:, :, :, D:D + 1],
                    ones_f[:, 0:1].to_broadcast((P, KB, 2, 1)),
                )
                for kg in range(2):
                    tp2 = ps_proj.tile([P, 1024], FP32, tag="w2")
                    for kbi in range(8):
                        kb = kg * 8 + kbi
                        # full-128 transpose: out[s, (h d)] for one k block
                        nc.tensor.transpose(
                            tp2[:, kbi * P:(kbi + 1) * P],
                            VT[:, kb * P:(kb + 1) * P].bitcast(FP32),
                            ident[:],
                        )
                    nc.vector.tensor_copy(
                        vx[:, kg * 8:(kg + 1) * 8, :, 0:D],
                        tp2[:].rearrange("pp (kbi h d) -> pp kbi h d", kbi=8, h=2),
                    )

                # ---------- attention
                z_sb = z_pool.tile([P, S], FP32R, tag="z")
                z_sbs.append(z_sb)
                for qj in range(QC):
                    nkb = 4 * qj + 4
                    zps = [
                        ps_z.tile([D + 1, 512], FP32, tag="z", name=f"zps{hh}")
                        for hh in range(2)
                    ]
                    for kb in range(nkb):
                        m = kb - 4 * qj
                        # causally-dead left columns skipped: restrict matmul/
                        # exp width to [c0:512) (c0 capped at 256 so fp32r
                        # stays at full rate, N >= 256)
                        c0 = 0 if m < 1 else min(P * m, 256)
                        nw = 512 - c0
                        # both heads' S^T in one 2-bank psum tile -> single exp
                        st = ps_work.tile([P, 1024], FP32, tag="w")
                        for h in range(2):
                            nc.tensor.matmul(
                                st[:, h * 512 + c0:(h + 1) * 512],
                                KT[h * D:(h + 1) * D, kb * P:(kb + 1) * P],
                                QT[h * D:(h + 1) * D,
                                   qj * 512 + c0:(qj + 1) * 512],
                                start=True,
                                stop=True,
                            )
                        pt = pt_pool.tile([P, 1024], FP32R, tag="pt")
                        if c0 > 0:
                            st3 = st[:].rearrange("pp (h c) -> pp h c", h=2)
                            pt3 = pt[:].rearrange("pp (h c) -> pp h c", h=2)
                            nc.scalar.activation(
                                pt3[:, :, c0:512], st3[:, :, c0:512], EXP,
                                scale=0.125,
                            )
                        else:
                            nc.scalar.activation(pt[:], st[:], EXP, scale=0.125)
                        if m >= 0:
                            # zero everything left of the diagonal in [c0:512)
                            w0 = P * m
                            for h in range(2):
                                nc.gpsimd.affine_select(
                                    out=pt[:, h * 512 + c0:h * 512 + w0 + P],
                                    in_=pt[:, h * 512 + c0:h * 512 + w0 + P],
                                    compare_op=mybir.AluOpType.is_ge,
                                    fill=0.0,
                                    base=-(w0 - c0),
                                    pattern=[[1, w0 + P - c0]],
                                    channel_multiplier=-1,
                                )
                        for h in range(2):
                            nc.tensor.matmul(
                                zps[h][:, c0:512],
                                vx[:, kb, h, :],
                                pt[:, h * 512 + c0:(h + 1) * 512],
                                start=(kb == 0),
                                stop=(kb == nkb - 1),
                            )
                    # normalize by the denominator (row 64 of zps):
                    # reciprocal, DMA partition-broadcast, multiply
                    zsl = slice(qj * 512, (qj + 1) * 512)
                    for h in range(2):
                        rcp = rc_pool.tile([D + 1, 512], FP32, tag="rc")
                        nc.vector.reciprocal(rcp[D:D + 1, :], zps[h][D:D + 1, :])
                        Rs = rc_pool.tile([D, 512], FP32, tag="rs")
                        nc.sync.dma_start(
                            Rs[:],
                            rcp[D:D + 1, None, :].to_broadcast((1, D, 512)),
                        )
                        if h == 0:
                            nc.vector.tensor_mul(
                                z_sb[0:D, zsl], zps[h][0:D, :], Rs[:]
                            )
                        else:
                            ztmp = zt_pool.tile([D, 512], FP32R, tag="zt")
                            nc.vector.tensor_mul(ztmp[:], zps[h][0:D, :], Rs[:])
                            nc.sync.dma_start(z_sb[64:128, zsl], ztmp[:])

        # ---------- output: O[q, m] = sum_p Z2h^T.T @ Wo2h
        with (
            tc.tile_pool(name="pso", bufs=2, space="PSUM") as ps_o,
            tc.tile_pool(name="osb", bufs=3) as o_pool,
        ):
            for qb in range(KB):
                po = ps_o.tile([P, 1024], FP32, tag="o")
                for mj in range(2):
                    for p in range(NP):
                        nc.tensor.matmul(
                            po[:, mj * 512:(mj + 1) * 512],
                            z_sbs[p][:, qb * P:(qb + 1) * P],
                            wo_rs[p][:, mj * 512:(mj + 1) * 512],
                            start=(p == 0),
                            stop=(p == NP - 1),
                        )
                ob = o_pool.tile([P, 1024], FP32, tag="o")
                nc.vector.tensor_copy(ob[:], po[:])
                nc.sync.dma_start(out_d[qb * P:(qb + 1) * P, :], ob[:])


_NC_CACHE = None


def _build_nc(split_waits=True):
    global _NC_CACHE
    if _NC_CACHE is not None and split_waits:
        return _NC_CACHE
    nc = bass.Bass("TRN2", target_bir_lowering=False, debug=False, num_devices=8)
    resid_d = nc.dram_tensor("resid", [S, M], FP32, kind="ExternalInput").ap()
    wq_d = nc.dram_tensor("wq", [H // 2, M, D], FP32, kind="ExternalInput").ap()
    wk_d = nc.dram_tensor("wk", [H // 2, M, D], FP32, kind="ExternalInput").ap()
    wv_d = nc.dram_tensor("wv", [H // 2, M, D], FP32, kind="ExternalInput").ap()
    wo_d = nc.dram_tensor("wo", [H // 2, D, M], FP32, kind="ExternalInput").ap()
    out_d = nc.dram_tensor("out", [S, M], FP32, kind="ExternalOutput").ap()
    with tile.TileContext(nc) as tc:
        _body(tc, nc, resid_d, wq_d, wk_d, wv_d, wo_d, out_d)
    if split_waits:
        _split_multiwait_instructions(nc)
        _NC_CACHE = nc
    return nc


def run(resid, w_q, w_k, w_v, w_o, **spmd_kwargs):
    """Build + run on 8 cores; returns (full output [4,2048,1024], BassKernelResults)."""
    resid = np.asarray(resid, dtype=np.float32)
    w_q = np.asarray(w_q, dtype=np.float32)
    w_k = np.asarray(w_k, dtype=np.float32)
    w_v = np.asarray(w_v, dtype=np.float32)
    w_o = np.asarray(w_o, dtype=np.float32)

    nc = _build_nc()
    in_maps = []
    for c in range(8):
        b, hh = c // 2, c % 2
        hs = slice(8 * hh, 8 * hh + 8)
        in_maps.append(
            {
                "resid": np.ascontiguousarray(resid[b]),
                "wq": np.ascontiguousarray(w_q[hs]),
                "wk": np.ascontiguousarray(w_k[hs]),
                "wv": np.ascontiguousarray(w_v[hs]),
                "wo": np.ascontiguousarray(w_o[hs]),
            }
        )
    res = run_bass_kernel_spmd(nc, in_maps, core_ids=list(range(8)), **spmd_kwargs)
    outs = [r["out"] for r in res.results]
    full = np.stack([outs[2 * b] + outs[2 * b + 1] for b in range(B)])
    return full.astype(np.float32), res


def kernel(resid, w_q, w_k, w_v, w_o):
    full, _ = run(resid, w_q, w_k, w_v, w_o)
    return full

